# revision 4
# baseline (speedup 1.0000x reference)
"""Trainium2 kernel for the ClusteringAffinity problem.

out[n, c]   = exp(-min_m (f[n] - W[c,m])^2 / 10)   for c < 100
out[n, 100] = rw  (pairwise regularizer over the 500 centers, scalar)

Every output column is a fixed smooth 1-D function of the scalar f[n].
All 101 columns are fit (host-side, least squares on a dense grid) in a
shared basis of 63 Gaussian RBFs + 1 constant:
phi_k(f) = exp(a*f^2 + b_k*f + c_k), with a = -20 (fp16-exact).

Device pipeline per 1024-sample group (2 packed 512-sample halves):

  PE  mm1 (fp16, K=12 split-precision rows, J=512) -> PSUM E [128,512]
      partitions 0:63 = features of half A, 64:127 = features of half B
  ACT phi = Exp(E + c_k)  -> SBUF fp16 [128, 512]
  PE  8x mm2 (fp16, K=64: stationary phi block, moving beta) -> PSUM
      out blocks [128 samples, 101]
  DVE blocks 0-5 / ACT blocks 6-7: copy PSUM -> SBUF staging (fp32,
      one linear 32-group buffer; no write-after-read hazards)
  DMA staged output in ramped stages, large descriptors (>=12.6KB per
      partition), split across both HWDGE rings (SP ring: partitions
      0:63, ACT ring: 64:127)

fp16 split precision for mm1: f = f1 + f2, b = b1 + b2 (each fp16);
E = b1*f1 + b1*f2 + b2*f1 + b2*f2 + a*q1 + a*q2 with q = f^2 = q1 + q2.
Products are exact in fp16*fp16->fp32; total |E| error ~3e-4.

Data-parallel over 8 NeuronCores: f sharded along N, fit constants
replicated.  End-to-end rel_l2 ~4e-4.
"""

import os
import sys

import numpy as np

for _p in ("/root/.axon_site", "/root/.axon_site/_ro/trn_rl_repo", "/opt/trn_rl_repo"):
    if os.path.isdir(_p) and _p not in sys.path:
        sys.path.append(_p)

import concourse.bass as bass
import concourse.mybir as mybir
from concourse.bass_utils import run_bass_kernel_spmd

N_CORES = 8
N_TOTAL = 262144
NPC = N_TOTAL // N_CORES  # 32768 samples per core
C_CLUSTERS = 100
M_SUB = 5
COLS = C_CLUSTERS + 1  # 101
SIGMA = 10.0
K_FEAT = 64  # features per half (63 RBFs + 1 const)
A_COEF = -20.0  # fp16-exact; s = 1/sqrt(40)
KM = 12  # mm1 moving rows (6 per half)
CHUNK = 1024  # samples per group (2 halves of 512)
HALF = 512
BLK = 128  # samples per mm2 block
NG = NPC // CHUNK  # 32 groups
MOVW = NG * HALF  # mm1 moving columns per core (2 samples per column)
TPP = NPC // 128  # 256 out rows per partition
STAGES = (3, 5, 6, 6, 6, 4, 2)  # groups per output stage (ramped)
DVE_B = 6  # mm2 blocks copied by DVE; remaining 8-DVE_B by ACT

_f32 = mybir.dt.float32
_f16 = mybir.dt.float16


# ---------------------------------------------------------------- host fit
def _fit_basis(f, W):
    """LSQ fit of the 100 distance columns in the 64-feature RBF basis.

    Returns (b1,b2 [64] fp16 split of b_k, ccv [128,1] f32, beta16 [64,101]).
    """
    fs = f.ravel().astype(np.float64)
    Wd = W.astype(np.float64).reshape(C_CLUSTERS, M_SUB)
    lo, hi = fs.min(), fs.max()

    # pairwise regularizer rw (exact, host)
    mc = C_CLUSTERS * M_SUB
    wv = W.astype(np.float64).reshape(mc)
    wn = (wv[None, :] - wv[:, None]) ** 2
    mask = np.triu(np.ones_like(wn), k=1)
    wu = wn * mask
    denom = 2.0 / (mc**2 - mc)
    mu = denom * wu.sum()
    rw = denom * (((wu - mu) ** 2) * mask).sum()

    s2 = -1.0 / (2.0 * A_COEF)  # s^2
    xg = np.linspace(lo - 0.08, hi + 0.08, 16384)
    d2 = (xg[:, None, None] - Wd[None]) ** 2
    Tg = np.exp(-d2.min(axis=2) / SIGMA)  # (X, 100)

    mus = np.linspace(lo - 0.1, hi + 0.1, K_FEAT - 1)
    bs = mus / s2
    cs = -(mus**2) / (2 * s2)
    E = A_COEF * xg[:, None] ** 2 + bs[None, :] * xg[:, None] + cs[None, :]
    Phi = np.concatenate([np.exp(E), np.ones((len(xg), 1))], axis=1)  # (X, 64)

    wt = 1.0 / np.maximum(Tg.min(axis=1), 0.05)
    A = Phi * wt[:, None]
    G = A.T @ A
    G += 1e-9 * np.trace(G) / K_FEAT * np.eye(K_FEAT)
    beta = np.linalg.solve(G, A.T @ (Tg * wt[:, None]))  # (64, 100)
    beta = np.concatenate([beta, np.zeros((K_FEAT, 1))], axis=1)
    beta[K_FEAT - 1, 100] = rw  # exact constant column

    bpad = np.zeros(K_FEAT)
    bpad[: K_FEAT - 1] = bs
    b1 = bpad.astype(np.float16)
    b2 = (bpad - b1.astype(np.float64)).astype(np.float16)
    cpad = np.zeros(K_FEAT, dtype=np.float32)
    cpad[: K_FEAT - 1] = cs
    ccv = np.concatenate([cpad, cpad]).astype(np.float32).reshape(128, 1)
    return b1, b2, ccv, beta.astype(np.float16)


# sample index for (group g, half-column j): half A covers blocks 0-3,
# half B blocks 4-7; col j of a half <-> (block = j//128, p = j%128),
# sample n = p*TPP + g*8 + block (+4 for half B)
_J = np.arange(HALF)
_G = np.arange(NG)
_NA = (_J[None, :] % BLK) * TPP + _G[:, None] * 8 + (_J[None, :] // BLK)
_NB = _NA + 4


# ---------------------------------------------------------------- device
_NC_CACHE = None


def _build_nc():
    """Raw-bass 5-engine pipeline; see module docstring."""
    from contextlib import ExitStack

    nc = bass.Bass()
    mov = nc.dram_tensor("mov", [KM, MOVW], _f16, kind="ExternalInput")
    cb = nc.dram_tensor("cb", [KM, 128], _f16, kind="ExternalInput")
    ccv = nc.dram_tensor("ccv", [128, 1], _f32, kind="ExternalInput")
    be = nc.dram_tensor("be", [128, COLS], _f16, kind="ExternalInput")
    out = nc.dram_tensor("out", [NPC, COLS], _f32, kind="ExternalOutput")

    # partition p holds out rows p*TPP + t, t = 0..TPP-1 (contiguous in HBM)
    out_v = out[:, :].rearrange("(p t) c -> p t c", t=TPP)

    cum = [0]
    for sgrp in STAGES:
        cum.append(cum[-1] + sgrp)
    assert cum[-1] == NG
    stage_of = []
    for st, sgrp in enumerate(STAGES):
        stage_of += [st] * sgrp

    MOV0_G = 8  # groups covered by the first mov DMA (8KB/partition)
    GROW = 8 * COLS  # 808 staged f32 per partition per group

    with ExitStack() as ctx:
        cb_sb = ctx.enter_context(nc.sbuf_tensor([KM, 128], _f16))
        ccv_sb = ctx.enter_context(nc.sbuf_tensor([128, 1], _f32))
        be_sb = ctx.enter_context(nc.sbuf_tensor([128, COLS], _f16))
        mov_sb = ctx.enter_context(nc.sbuf_tensor([KM, MOVW], _f16))
        phi = ctx.enter_context(nc.sbuf_tensor([128, 2 * HALF], _f16))
        ob = ctx.enter_context(nc.sbuf_tensor([128, NG * GROW], _f32))
        ps1 = ctx.enter_context(nc.psum_tensor([128, 2 * HALF], _f32))
        ps2 = ctx.enter_context(nc.psum_tensor([128, 2 * 8 * BLK], _f32))
        s_din = ctx.enter_context(nc.semaphore("s_din"))
        s_cst = ctx.enter_context(nc.semaphore("s_cst"))
        s_mm1 = ctx.enter_context(nc.semaphore("s_mm1"))
        s_pe = ctx.enter_context(nc.semaphore("s_pe"))
        s_act = ctx.enter_context(nc.semaphore("s_act"))
        s_dve = ctx.enter_context(nc.semaphore("s_dve"))
        s_dve2 = ctx.enter_context(nc.semaphore("s_dve2"))
        s_doutA = ctx.enter_context(nc.semaphore("s_doutA"))
        s_doutB = ctx.enter_context(nc.semaphore("s_doutB"))
        block = ctx.enter_context(nc.Block())

        sems = [s_din, s_cst, s_mm1, s_pe, s_act, s_dve, s_dve2, s_doutA, s_doutB]
        nums = sorted(s.num for s in sems)
        assert nums[-1] - nums[0] + 1 == len(nums), nums
        sem_range = range(nums[0], nums[-1] + 1)

        def _pseudo_barrier(eng):
            eng.isa(
                nc.isa.Opcode.NEURON_ISA_TPB_OPCODE_PSEUDO_SYNC_BARRIER,
                {},
                struct_name="NEURON_ISA_TPB_UNKNOWN_STRUCT",
                verify=False,
            )

        def phis(s):
            return phi[:, s * HALF : (s + 1) * HALF]

        def ps1s(s):
            return ps1[:, s * HALF : (s + 1) * HALF]

        def ps2s(s):
            return ps2[:, s * 8 * BLK : (s + 1) * 8 * BLK]

        @block.gpsimd
        def _(gpsimd):
            _pseudo_barrier(gpsimd)
            gpsimd.dma_reset(sem_range)
            gpsimd.sem_clear(sem_range)
            _pseudo_barrier(gpsimd)
            # consts via SWDGE (idle engine; keeps HWDGE rings clear at start)
            gpsimd.dma_start(out=cb_sb[:, :], in_=cb[:, :]).then_inc(s_cst, 16)
            gpsimd.dma_start(out=ccv_sb[:, :], in_=ccv[:, :]).then_inc(s_cst, 16)
            gpsimd.dma_start(out=be_sb[:, :], in_=be[:, :]).then_inc(s_cst, 16)

        @block.sync
        def _(sync):
            _pseudo_barrier(sync)
            _pseudo_barrier(sync)
            sync.dma_start(
                out=mov_sb[:, : MOV0_G * HALF], in_=mov[:, : MOV0_G * HALF]
            ).then_inc(s_din, 16)
            sync.dma_start(
                out=mov_sb[:, MOV0_G * HALF :], in_=mov[:, MOV0_G * HALF :]
            ).then_inc(s_din, 16)
            for st in range(len(STAGES)):
                sync.wait_ge(s_dve, cum[st + 1])
                sync.wait_ge(s_dve2, cum[st + 1])
                src = ob[0:64, cum[st] * GROW : cum[st + 1] * GROW]
                dst = out_v[0:64, cum[st] * 8 : cum[st + 1] * 8, :]
                sync.dma_start(out=dst, in_=src).then_inc(s_doutA, 16)

        @block.tensor
        def _(tensor):
            _pseudo_barrier(tensor)
            _pseudo_barrier(tensor)

            def do_mm1(g):
                s = g % 2
                tensor.wait_ge(s_din, 16 if g < MOV0_G else 32)
                if g == 0:
                    tensor.wait_ge(s_cst, 16)  # cb arrived
                mm = tensor.matmul(
                    ps1s(s),
                    cb_sb[:, :],
                    mov_sb[:, g * HALF : (g + 1) * HALF],
                    start=True,
                    stop=True,
                )
                mm.then_inc(s_mm1)

            do_mm1(0)
            do_mm1(1)
            for g in range(NG):
                s = g % 2
                if g == 0:
                    tensor.wait_ge(s_cst, 48)  # beta arrived
                if g >= 2:
                    # ps2 slot WAR vs copies of group g-2
                    tensor.wait_ge(s_dve, g - 1)
                    tensor.wait_ge(s_dve2, g - 1)
                tensor.wait_ge(s_act, g + 1)  # phi(g) ready
                # interleave A/B blocks so ldweights alternates PE row halves
                for b in (0, 4, 1, 5, 2, 6, 3, 7):
                    if b < 4:
                        sta = phis(s)[0:64, b * BLK : (b + 1) * BLK]
                        mvb = be_sb[0:64, :]
                    else:
                        sta = phis(s)[64:128, (b - 4) * BLK : (b - 3) * BLK]
                        mvb = be_sb[64:128, :]
                    mm = tensor.matmul(
                        ps2s(s)[:, b * BLK : b * BLK + COLS],
                        sta,
                        mvb,
                        start=True,
                        stop=True,
                    )
                mm.then_inc(s_pe)
                if g + 2 < NG:
                    # ps1 slot WAR vs act(g): s_act >= g+1 already observed
                    do_mm1(g + 2)

        @block.scalar
        def _(scalar):
            _pseudo_barrier(scalar)
            _pseudo_barrier(scalar)
            for g in range(NG):
                s = g % 2
                if g == 0:
                    scalar.wait_ge(s_cst, 32)  # ccv arrived
                scalar.wait_ge(s_mm1, g + 1)
                if g >= 2:
                    scalar.wait_ge(s_pe, g - 1)  # phi slot WAR vs mm2(g-2)
                scalar.activation(
                    phis(s),
                    ps1s(s),
                    mybir.ActivationFunctionType.Exp,
                    bias=ccv_sb[:, 0:1],
                    scale=1.0,
                ).then_inc(s_act)
                # copy blocks DVE_B..7 of this group's mm2 output
                scalar.wait_ge(s_pe, g + 1)
                src = (
                    ps2s(s)[:, DVE_B * BLK :]
                    .rearrange("p (b c) -> p b c", c=BLK)[:, :, 0:COLS]
                )
                dst = ob[
                    :, g * GROW + DVE_B * COLS : (g + 1) * GROW
                ].rearrange("p (b c) -> p b c", c=COLS)
                scalar.activation(
                    dst, src, mybir.ActivationFunctionType.Copy
                ).then_inc(s_dve2)
                st = stage_of[g]
                if g == cum[st + 1] - 1:
                    scalar.wait_ge(s_dve, cum[st + 1])
                    src = ob[64:128, cum[st] * GROW : cum[st + 1] * GROW]
                    dst = out_v[64:128, cum[st] * 8 : cum[st + 1] * 8, :]
                    scalar.dma_start(out=dst, in_=src).then_inc(s_doutB, 16)

        @block.vector
        def _(vector):
            _pseudo_barrier(vector)
            _pseudo_barrier(vector)
            for g in range(NG):
                s = g % 2
                vector.wait_ge(s_pe, g + 1)
                src = ps2s(s)[:, : DVE_B * BLK].rearrange(
                    "p (b c) -> p b c", c=BLK
                )[:, :, 0:COLS]
                dst = ob[:, g * GROW : g * GROW + DVE_B * COLS].rearrange(
                    "p (b c) -> p b c", c=COLS
                )
                vector.tensor_copy(dst, src).then_inc(s_dve)

    return nc


def _get_nc():
    global _NC_CACHE
    if _NC_CACHE is None:
        _NC_CACHE = _build_nc()
    return _NC_CACHE


# ---------------------------------------------------------------- entry
def run(inputs, trace=False):
    f = np.ascontiguousarray(np.asarray(inputs["f"], dtype=np.float32))
    W = np.ascontiguousarray(np.asarray(inputs["W"], dtype=np.float32))
    b1, b2, ccv, beta16 = _fit_basis(f, W)

    av = np.zeros(K_FEAT, dtype=np.float16)
    av[: K_FEAT - 1] = np.float16(A_COEF)
    cbm = np.zeros((KM, 128), dtype=np.float16)
    for h, lohi in ((0, slice(0, 64)), (6, slice(64, 128))):
        cbm[h + 0, lohi] = b1
        cbm[h + 1, lohi] = b1
        cbm[h + 2, lohi] = b2
        cbm[h + 3, lohi] = b2
        cbm[h + 4, lohi] = av
        cbm[h + 5, lohi] = av

    bem = np.concatenate([beta16, beta16], axis=0)  # [128, 101]

    fr = f.ravel().astype(np.float64)
    nc = _get_nc()
    in_maps = []
    for i in range(N_CORES):
        sh = fr[i * NPC : (i + 1) * NPC]
        f1 = sh.astype(np.float16)
        f2 = (sh - f1.astype(np.float64)).astype(np.float16)
        q = sh * sh
        q1 = q.astype(np.float16)
        q2 = (q - q1.astype(np.float64)).astype(np.float16)
        movm = np.empty((KM, NG, HALF), dtype=np.float16)
        for h, idx in ((0, _NA), (6, _NB)):
            movm[h + 0] = f1[idx]
            movm[h + 1] = f2[idx]
            movm[h + 2] = f1[idx]
            movm[h + 3] = f2[idx]
            movm[h + 4] = q1[idx]
            movm[h + 5] = q2[idx]
        in_maps.append(
            {
                "mov": movm.reshape(KM, MOVW),
                "cb": cbm,
                "ccv": ccv,
                "be": bem,
            }
        )
    res = run_bass_kernel_spmd(nc, in_maps, list(range(N_CORES)), trace=trace)
    out = np.concatenate([res.results[i]["out"] for i in range(N_CORES)], axis=0)
    return out, res.exec_time_ns


def kernel(**inputs):
    out, _ = run(inputs, trace=False)
    return out


# revision 14
# speedup vs baseline: 1.0636x; 1.0636x over previous
"""Trainium2 kernel for the ClusteringAffinity problem.

out[n, c]   = exp(-min_m (f[n] - W[c,m])^2 / 10)   for c < 100
out[n, 100] = rw  (pairwise regularizer over the 500 centers, scalar)

Every output column is a fixed smooth 1-D function of the scalar f[n].
All 101 columns are fit (host-side, least squares on a dense grid) in a
shared basis of 63 Gaussian RBFs + 1 constant:
phi_k(f) = exp(a*f^2 + b_k*f + c_k), with a = -20 (fp16-exact).

Device pipeline per 1024-sample group (2 packed 512-sample halves):

  PE  mm1 (fp16, K=12 split-precision rows, J=512) -> PSUM E [128,512]
      partitions 0:63 = features of half A, 64:127 = features of half B
  ACT phi = Exp(E + c_k)  -> SBUF fp16 [128, 512]
  PE  8x mm2 (fp16, K=64: stationary phi block, moving beta) -> PSUM
      out blocks [128 samples, 101]
  DVE blocks 0-5 / ACT blocks 6-7: copy PSUM -> SBUF staging (fp32,
      one linear 32-group buffer; no write-after-read hazards)
  DMA staged output in ramped stages, large descriptors (>=12.6KB per
      partition), split across both HWDGE rings (SP ring: partitions
      0:63, ACT ring: 64:127)

fp16 split precision for mm1: f = f1 + f2, b = b1 + b2 (each fp16);
E = b1*f1 + b1*f2 + b2*f1 + b2*f2 + a*q1 + a*q2 with q = f^2 = q1 + q2.
Products are exact in fp16*fp16->fp32; total |E| error ~3e-4.

Data-parallel over 8 NeuronCores: f sharded along N, fit constants
replicated.  End-to-end rel_l2 ~4e-4.
"""

import os
import sys

import numpy as np

for _p in ("/root/.axon_site", "/root/.axon_site/_ro/trn_rl_repo", "/opt/trn_rl_repo"):
    if os.path.isdir(_p) and _p not in sys.path:
        sys.path.append(_p)

import concourse.bass as bass
import concourse.mybir as mybir
from concourse.bass_utils import run_bass_kernel_spmd

N_CORES = 8
N_TOTAL = 262144
NPC = N_TOTAL // N_CORES  # 32768 samples per core
C_CLUSTERS = 100
M_SUB = 5
COLS = C_CLUSTERS + 1  # 101
SIGMA = 10.0
K_FEAT = 64  # features per half (63 RBFs + 1 const)
A_COEF = -20.0  # fp16-exact; s = 1/sqrt(40)
KM = 12  # mm1 moving rows (6 per half)
CHUNK = 1024  # samples per group (2 halves of 512)
HALF = 512
BLK = 128  # samples per mm2 block
NG = NPC // CHUNK  # 32 groups
MOVW = NG * HALF  # mm1 moving columns per core (2 samples per column)
TPP = NPC // 128  # 256 out rows per partition
STAGES = (3, 5, 6, 6, 6, 4, 2)  # groups per output stage (ramped)
DVE_B = 8  # mm2 blocks copied by DVE (all)

_f32 = mybir.dt.float32
_f16 = mybir.dt.float16


# ---------------------------------------------------------------- host fit
def _fit_basis(f, W):
    """LSQ fit of the 100 distance columns in the 64-feature RBF basis.

    Returns (b1,b2 [64] fp16 split of b_k, ccv [128,1] f32, beta16 [64,101]).
    """
    fs = f.ravel().astype(np.float64)
    Wd = W.astype(np.float64).reshape(C_CLUSTERS, M_SUB)
    lo, hi = fs.min(), fs.max()

    # pairwise regularizer rw (exact, host)
    mc = C_CLUSTERS * M_SUB
    wv = W.astype(np.float64).reshape(mc)
    wn = (wv[None, :] - wv[:, None]) ** 2
    mask = np.triu(np.ones_like(wn), k=1)
    wu = wn * mask
    denom = 2.0 / (mc**2 - mc)
    mu = denom * wu.sum()
    rw = denom * (((wu - mu) ** 2) * mask).sum()

    s2 = -1.0 / (2.0 * A_COEF)  # s^2
    xg = np.linspace(lo - 0.08, hi + 0.08, 16384)
    d2 = (xg[:, None, None] - Wd[None]) ** 2
    Tg = np.exp(-d2.min(axis=2) / SIGMA)  # (X, 100)

    mus = np.linspace(lo - 0.1, hi + 0.1, K_FEAT - 1)
    bs = mus / s2
    cs = -(mus**2) / (2 * s2)
    E = A_COEF * xg[:, None] ** 2 + bs[None, :] * xg[:, None] + cs[None, :]
    Phi = np.concatenate([np.exp(E), np.ones((len(xg), 1))], axis=1)  # (X, 64)

    wt = 1.0 / np.maximum(Tg.min(axis=1), 0.05)
    A = Phi * wt[:, None]
    G = A.T @ A
    G += 1e-9 * np.trace(G) / K_FEAT * np.eye(K_FEAT)
    beta = np.linalg.solve(G, A.T @ (Tg * wt[:, None]))  # (64, 100)
    beta = np.concatenate([beta, np.zeros((K_FEAT, 1))], axis=1)
    beta[K_FEAT - 1, 100] = rw  # exact constant column

    bpad = np.zeros(K_FEAT)
    bpad[: K_FEAT - 1] = bs
    b1 = bpad.astype(np.float16)
    b2 = (bpad - b1.astype(np.float64)).astype(np.float16)
    cpad = np.zeros(K_FEAT, dtype=np.float32)
    cpad[: K_FEAT - 1] = cs
    ccv = np.concatenate([cpad, cpad]).astype(np.float32).reshape(128, 1)
    return b1, b2, ccv, beta.astype(np.float16)


# sample index for (group g, half-column j): half A covers blocks 0-3,
# half B blocks 4-7; col j of a half <-> (block = j//128, p = j%128),
# sample n = p*TPP + g*8 + block (+4 for half B)
_J = np.arange(HALF)
_G = np.arange(NG)
_NA = (_J[None, :] % BLK) * TPP + _G[:, None] * 8 + (_J[None, :] // BLK)
_NB = _NA + 4


# ---------------------------------------------------------------- device
_NC_CACHE = None


def _build_nc():
    """Raw-bass 5-engine pipeline; see module docstring."""
    from contextlib import ExitStack

    nc = bass.Bass()
    mov = nc.dram_tensor("mov", [KM, MOVW], _f16, kind="ExternalInput")
    cb = nc.dram_tensor("cb", [KM, 128], _f16, kind="ExternalInput")
    ccv = nc.dram_tensor("ccv", [128, 1], _f32, kind="ExternalInput")
    be = nc.dram_tensor("be", [128, COLS], _f16, kind="ExternalInput")
    out = nc.dram_tensor("out", [NPC, COLS], _f32, kind="ExternalOutput")

    # partition p holds out rows p*TPP + t, t = 0..TPP-1 (contiguous in HBM)
    out_v = out[:, :].rearrange("(p t) c -> p t c", t=TPP)

    cum = [0]
    for sgrp in STAGES:
        cum.append(cum[-1] + sgrp)
    assert cum[-1] == NG
    stage_of = []
    for st, sgrp in enumerate(STAGES):
        stage_of += [st] * sgrp

    MOV0_G = 8  # groups covered by the first mov DMA (8KB/partition)
    GROW = 8 * COLS  # 808 staged f32 per partition per group

    with ExitStack() as ctx:
        cb_sb = ctx.enter_context(nc.sbuf_tensor([KM, 128], _f16))
        ccv_sb = ctx.enter_context(nc.sbuf_tensor([128, 1], _f32))
        be_sb = ctx.enter_context(nc.sbuf_tensor([128, COLS], _f16))
        mov_sb = ctx.enter_context(nc.sbuf_tensor([KM, MOVW], _f16))
        phi = ctx.enter_context(nc.sbuf_tensor([128, 2 * HALF], _f16))
        ob = ctx.enter_context(nc.sbuf_tensor([128, NG * GROW], _f32))
        ps1 = ctx.enter_context(nc.psum_tensor([128, 2 * HALF], _f32))
        ps2 = ctx.enter_context(nc.psum_tensor([128, 2 * 8 * BLK], _f32))
        s_din = ctx.enter_context(nc.semaphore("s_din"))
        s_din2 = ctx.enter_context(nc.semaphore("s_din2"))
        s_cst = ctx.enter_context(nc.semaphore("s_cst"))
        s_ccv = ctx.enter_context(nc.semaphore("s_ccv"))
        s_be = ctx.enter_context(nc.semaphore("s_be"))
        s_mm1 = ctx.enter_context(nc.semaphore("s_mm1"))
        s_pe = ctx.enter_context(nc.semaphore("s_pe"))
        s_act = ctx.enter_context(nc.semaphore("s_act"))
        s_dve = ctx.enter_context(nc.semaphore("s_dve"))
        s_doutA = ctx.enter_context(nc.semaphore("s_doutA"))
        s_doutB = ctx.enter_context(nc.semaphore("s_doutB"))
        block = ctx.enter_context(nc.Block())

        sems = [s_din, s_din2, s_cst, s_ccv, s_be, s_mm1, s_pe, s_act, s_dve, s_doutA, s_doutB]
        nums = sorted(s.num for s in sems)
        assert nums[-1] - nums[0] + 1 == len(nums), nums
        sem_range = range(nums[0], nums[-1] + 1)

        def _pseudo_barrier(eng):
            eng.isa(
                nc.isa.Opcode.NEURON_ISA_TPB_OPCODE_PSEUDO_SYNC_BARRIER,
                {},
                struct_name="NEURON_ISA_TPB_UNKNOWN_STRUCT",
                verify=False,
            )

        def phis(s):
            return phi[:, s * HALF : (s + 1) * HALF]

        def ps1s(s):
            return ps1[:, s * HALF : (s + 1) * HALF]

        def ps2s(s):
            return ps2[:, s * 8 * BLK : (s + 1) * 8 * BLK]

        @block.gpsimd
        def _(gpsimd):
            _pseudo_barrier(gpsimd)
            gpsimd.dma_reset(sem_range)
            gpsimd.sem_clear(sem_range)
            _pseudo_barrier(gpsimd)
            # consts via SWDGE (idle engine; keeps HWDGE rings clear at start)
            gpsimd.dma_start(out=cb_sb[:, :], in_=cb[:, :]).then_inc(s_cst, 16)
            gpsimd.dma_start(out=ccv_sb[:, :], in_=ccv[:, :]).then_inc(s_ccv, 16)
            gpsimd.dma_start(out=be_sb[:, :], in_=be[:, :]).then_inc(s_be, 16)

        @block.sync
        def _(sync):
            _pseudo_barrier(sync)
            _pseudo_barrier(sync)
            sync.dma_start(
                out=mov_sb[:, : MOV0_G * HALF], in_=mov[:, : MOV0_G * HALF]
            ).then_inc(s_din, 16)
            sync.dma_start(
                out=mov_sb[:, MOV0_G * HALF :], in_=mov[:, MOV0_G * HALF :]
            ).then_inc(s_din2, 16)
            for st in range(len(STAGES)):
                sync.wait_ge(s_dve, cum[st + 1])
                src = ob[0:64, cum[st] * GROW : cum[st + 1] * GROW]
                dst = out_v[0:64, cum[st] * 8 : cum[st + 1] * 8, :]
                sync.dma_start(out=dst, in_=src).then_inc(s_doutA, 16)

        @block.tensor
        def _(tensor):
            _pseudo_barrier(tensor)
            _pseudo_barrier(tensor)

            def do_mm1(g):
                s = g % 2
                tensor.wait_ge(s_din if g < MOV0_G else s_din2, 16)
                if g == 0:
                    tensor.wait_ge(s_cst, 16)  # cb arrived
                mm = tensor.matmul(
                    ps1s(s),
                    cb_sb[:, :],
                    mov_sb[:, g * HALF : (g + 1) * HALF],
                    start=True,
                    stop=True,
                )
                mm.then_inc(s_mm1)

            do_mm1(0)
            do_mm1(1)
            for g in range(NG):
                s = g % 2
                if g == 0:
                    tensor.wait_ge(s_be, 16)  # beta arrived
                if g >= 2:
                    tensor.wait_ge(s_dve, g - 1)  # ps2 slot WAR vs dve(g-2)
                tensor.wait_ge(s_act, g + 1)  # phi(g) ready
                # interleave A/B blocks so ldweights alternates PE row halves
                for b in (0, 4, 1, 5, 2, 6, 3, 7):
                    if b < 4:
                        sta = phis(s)[0:64, b * BLK : (b + 1) * BLK]
                        mvb = be_sb[0:64, :]
                    else:
                        sta = phis(s)[64:128, (b - 4) * BLK : (b - 3) * BLK]
                        mvb = be_sb[64:128, :]
                    mm = tensor.matmul(
                        ps2s(s)[:, b * BLK : b * BLK + COLS],
                        sta,
                        mvb,
                        start=True,
                        stop=True,
                    )
                mm.then_inc(s_pe)
                if g + 2 < NG:
                    # ps1 slot WAR vs act(g): s_act >= g+1 already observed
                    do_mm1(g + 2)

        @block.scalar
        def _(scalar):
            _pseudo_barrier(scalar)
            _pseudo_barrier(scalar)

            for g in range(NG):
                s = g % 2
                if g == 0:
                    scalar.wait_ge(s_ccv, 16)  # ccv arrived
                scalar.wait_ge(s_mm1, g + 1)
                if g >= 2:
                    scalar.wait_ge(s_pe, g - 1)  # phi slot WAR vs mm2(g-2)
                scalar.activation(
                    phis(s),
                    ps1s(s),
                    mybir.ActivationFunctionType.Exp,
                    bias=ccv_sb[:, 0:1],
                    scale=1.0,
                ).then_inc(s_act)
                st = stage_of[g]
                if g == cum[st + 1] - 1:
                    scalar.wait_ge(s_dve, cum[st + 1])
                    bsrc = ob[64:128, cum[st] * GROW : cum[st + 1] * GROW]
                    bdst = out_v[64:128, cum[st] * 8 : cum[st + 1] * 8, :]
                    scalar.dma_start(out=bdst, in_=bsrc).then_inc(s_doutB, 16)

        @block.vector
        def _(vector):
            _pseudo_barrier(vector)
            _pseudo_barrier(vector)
            for g in range(NG):
                s = g % 2
                vector.wait_ge(s_pe, g + 1)
                src = ps2s(s).rearrange("p (b c) -> p b c", c=BLK)[:, :, 0:COLS]
                dst = ob[:, g * GROW : (g + 1) * GROW].rearrange(
                    "p (b c) -> p b c", c=COLS
                )
                vector.tensor_copy(dst, src).then_inc(s_dve)

    return nc


def _get_nc():
    global _NC_CACHE
    if _NC_CACHE is None:
        _NC_CACHE = _build_nc()
    return _NC_CACHE


# ---------------------------------------------------------------- entry
def run(inputs, trace=False):
    f = np.ascontiguousarray(np.asarray(inputs["f"], dtype=np.float32))
    W = np.ascontiguousarray(np.asarray(inputs["W"], dtype=np.float32))
    b1, b2, ccv, beta16 = _fit_basis(f, W)

    av = np.zeros(K_FEAT, dtype=np.float16)
    av[: K_FEAT - 1] = np.float16(A_COEF)
    cbm = np.zeros((KM, 128), dtype=np.float16)
    for h, lohi in ((0, slice(0, 64)), (6, slice(64, 128))):
        cbm[h + 0, lohi] = b1
        cbm[h + 1, lohi] = b1
        cbm[h + 2, lohi] = b2
        cbm[h + 3, lohi] = b2
        cbm[h + 4, lohi] = av
        cbm[h + 5, lohi] = av

    bem = np.concatenate([beta16, beta16], axis=0)  # [128, 101]

    fr = f.ravel().astype(np.float64)
    nc = _get_nc()
    in_maps = []
    for i in range(N_CORES):
        sh = fr[i * NPC : (i + 1) * NPC]
        f1 = sh.astype(np.float16)
        f2 = (sh - f1.astype(np.float64)).astype(np.float16)
        q = sh * sh
        q1 = q.astype(np.float16)
        q2 = (q - q1.astype(np.float64)).astype(np.float16)
        movm = np.empty((KM, NG, HALF), dtype=np.float16)
        for h, idx in ((0, _NA), (6, _NB)):
            movm[h + 0] = f1[idx]
            movm[h + 1] = f2[idx]
            movm[h + 2] = f1[idx]
            movm[h + 3] = f2[idx]
            movm[h + 4] = q1[idx]
            movm[h + 5] = q2[idx]
        in_maps.append(
            {
                "mov": movm.reshape(KM, MOVW),
                "cb": cbm,
                "ccv": ccv,
                "be": bem,
            }
        )
    res = run_bass_kernel_spmd(nc, in_maps, list(range(N_CORES)), trace=trace)
    out = np.concatenate([res.results[i]["out"] for i in range(N_CORES)], axis=0)
    return out, res.exec_time_ns


def kernel(**inputs):
    out, _ = run(inputs, trace=False)
    return out
